# revision 29
# baseline (speedup 1.0000x reference)
"""Trainium2 Bass kernel for nn_DistributedExpert (dense transformer expert).

Computes, for x [4096, 2048]:
    h   = gelu(x @ fc1_w.T + fc1_b) @ fc2_w.T + fc2_b          (MLP branch)
    q/k/v = x @ {q,k,v}_w.T + b
    attn  = softmax(q @ k.T / sqrt(2048))
    out = (attn @ v) @ o_w.T + o_b + h

Distribution over 8 NeuronCores — everything is sequence-sharded (each core
owns 512 rows of x and of the output). Two collective-free launches:

  Launch 1: each core computes q/k/v (transposed layouts) for its rows.
  Host:     gathers the k/v shards (this replaces an on-device AllGather —
            measured: having ANY collective in the NEFF slows every matmul
            by ~21%, 216 -> 263 ns, so the gather is done on the host).
  Launch 2: MLP (full weights streamed, no expert sharding) + attention +
            output projection + combine.

  - Softmax uses the no-max-subtraction form (scores are O(1)); row sums are
    built with ones-matmuls on the transposed attention layout and the
    normalization is applied after the output projection.
  - Biases with an easy per-partition broadcast (q_b, k_b, fc1_b) are added
    on-device; v_b/o_b/fc2_b contributions are mathematically exact constant
    row-vectors, added on the host.

Matmuls run in bf16 (fp32 PSUM accumulation). All streamed weights are
pre-swizzled on the host into their exact SBUF image ([128 partitions x
contiguous free bytes]) so every weight DMA is a full-bandwidth linear copy.
"""

import os
import sys

sys.path.insert(0, "/opt/trn_rl_repo")

import numpy as np
import ml_dtypes

import concourse.bass as bass
import concourse.mybir as mybir
import concourse.tile as tile
from concourse import bacc
from concourse.bass_utils import run_bass_kernel_spmd

SEQ = 4096
HID = 2048
EXP = 8192
NCORES = 8
SSH = SEQ // NCORES   # 512 sequence rows per core
P = 128

HT = HID // P        # 16 hidden tiles
ET = EXP // P        # 64 expert tiles
ST = SSH // P        # 4 local-seq tiles
GT = SEQ // P        # 32 global-seq tiles

F32 = mybir.dt.float32
BF16 = mybir.dt.bfloat16
AF = mybir.ActivationFunctionType
BF_NP = ml_dtypes.bfloat16

_CACHE = {}


def _wslice(w_ap, blk, ntiles):
    # [128, ntiles, 512] SBUF-image slice for output-block `blk`
    sz = ntiles * 512
    return w_ap[:, blk * sz:(blk + 1) * sz].rearrange("p (a s) -> p a s", a=ntiles)


def _build_qkv():
    nc = bacc.Bacc("TRN2", target_bir_lowering=False, debug=False, num_devices=NCORES)
    xsh = nc.dram_tensor("xsh", [P, HT * SSH], BF16, kind="ExternalInput").ap()
    wq = nc.dram_tensor("wq", [P, 4 * HT * 512], BF16, kind="ExternalInput").ap()
    wk = nc.dram_tensor("wk", [P, 4 * HT * 512], BF16, kind="ExternalInput").ap()
    wv = nc.dram_tensor("wv", [P, 4 * HT * 512], BF16, kind="ExternalInput").ap()
    qb2 = nc.dram_tensor("qb2", [P, HT], F32, kind="ExternalInput").ap()
    kb2 = nc.dram_tensor("kb2", [P, HT], F32, kind="ExternalInput").ap()
    qT_o = nc.dram_tensor("qT_o", [P, HT * SSH], BF16, kind="ExternalOutput").ap()
    kT_o = nc.dram_tensor("kT_o", [P, HT * SSH], BF16, kind="ExternalOutput").ap()
    v_o = nc.dram_tensor("v_o", [P, ST * HID], BF16, kind="ExternalOutput").ap()

    with tile.TileContext(nc) as tc:
        with tc.tile_pool(name="const", bufs=1) as constp, \
             tc.tile_pool(name="st", bufs=1) as st, \
             tc.tile_pool(name="ws", bufs=3) as ws, \
             tc.tile_pool(name="ps", bufs=3, space="PSUM") as ps:
            qb_s = constp.tile([P, HT], F32)
            kb_s = constp.tile([P, HT], F32)
            nc.sync.dma_start(qb_s[:], qb2[:])
            nc.sync.dma_start(kb_s[:], kb2[:])
            xs = st.tile([P, HT, SSH], BF16)
            xsr = xsh.rearrange("p (a s) -> p a s", a=HT)
            for cch in range(4):
                nc.sync.dma_start(
                    xs[:, cch * 4:(cch + 1) * 4, :], xsr[:, cch * 4:(cch + 1) * 4, :]
                )
            qT = st.tile([P, HT, SSH], BF16)
            kT = st.tile([P, HT, SSH], BF16)
            v_s = st.tile([P, ST, HID], BF16)

            for dst, dst_o, w_ap, bias in (
                (qT, qT_o, wq, qb_s), (kT, kT_o, wk, kb_s)
            ):
                for g in range(4):
                    wt = ws.tile([P, HT, 512], BF16, name="wtile")
                    wsl = _wslice(w_ap, g, HT)
                    for cch in range(4):
                        nc.sync.dma_start(
                            wt[:, cch * 4:(cch + 1) * 4, :],
                            wsl[:, cch * 4:(cch + 1) * 4, :],
                        )
                    for m in range(4):
                        pt = ps.tile([P, SSH], F32, name="pA")
                        for k in range(HT):
                            nc.tensor.matmul(
                                pt[:], wt[:, k, m * P:(m + 1) * P], xs[:, k, :],
                                start=(k == 0), stop=(k == HT - 1),
                            )
                        nc.scalar.activation(
                            dst[:, g * 4 + m, :], pt[:], AF.Identity,
                            bias=bias[:, g * 4 + m:g * 4 + m + 1],
                        )
                    # stream this group's 4 tiles out while the next computes
                    nc.sync.dma_start(
                        _wslice(dst_o, g, 4), dst[:, g * 4:(g + 1) * 4, :]
                    )
            # v in natural layout [s_local, hid]  (v_b folded on host)
            for n in range(4):
                wt = ws.tile([P, HT, 512], BF16, name="wtile")
                nc.sync.dma_start(wt[:], _wslice(wv, n, HT))
                for m in range(ST):
                    pt = ps.tile([P, SSH], F32, name="pA")
                    for k in range(HT):
                        nc.tensor.matmul(
                            pt[:], xs[:, k, m * P:(m + 1) * P], wt[:, k, :],
                            start=(k == 0), stop=(k == HT - 1),
                        )
                    nc.vector.tensor_copy(v_s[:, m, n * 512:(n + 1) * 512], pt[:])
                nc.sync.dma_start(
                    v_o[:, n * 512::HID].rearrange("p (a s) -> p a s", a=ST)
                    if False else
                    v_o.rearrange("p (a s) -> p a s", a=ST)[:, :, n * 512:(n + 1) * 512],
                    v_s[:, :, n * 512:(n + 1) * 512],
                )
    nc.compile()
    return nc


def _build_main():
    nc = bacc.Bacc("TRN2", target_bir_lowering=False, debug=False, num_devices=NCORES)
    EXPSCALE = 1.0 / float(np.sqrt(np.float32(HID)))

    xsh = nc.dram_tensor("xsh", [P, HT * SSH], BF16, kind="ExternalInput").ap()
    qTi = nc.dram_tensor("qTi", [P, HT * SSH], BF16, kind="ExternalInput").ap()
    kT_all = nc.dram_tensor("kT_all", [NCORES * P, HT * SSH], BF16,
                            kind="ExternalInput").ap()
    v_all = nc.dram_tensor("v_all", [NCORES * P, ST * HID], BF16,
                           kind="ExternalInput").ap()
    wo = nc.dram_tensor("wo", [P, 4 * HT * 512], BF16, kind="ExternalInput").ap()
    w1 = nc.dram_tensor("w1", [P, 16 * HT * 512], BF16, kind="ExternalInput").ap()
    w2 = nc.dram_tensor("w2", [P, 4 * ET * 512], BF16, kind="ExternalInput").ap()
    b12 = nc.dram_tensor("b12", [P, ET], F32, kind="ExternalInput").ap()
    out = nc.dram_tensor("out", [SSH, HID], F32, kind="ExternalOutput").ap()

    with tile.TileContext(nc) as tc:
        with tc.tile_pool(name="const", bufs=1) as constp, \
             tc.tile_pool(name="persist", bufs=1) as persist:
            ones = constp.tile([P, 1], BF16)
            nc.vector.memset(ones[:], 1.0)
            b1_s = constp.tile([P, ET], F32)
            nc.sync.dma_start(b1_s[:], b12[:])

            xs = persist.tile([P, HT, SSH], BF16)
            xsr = xsh.rearrange("p (a s) -> p a s", a=HT)
            for cch in range(4):
                nc.sync.dma_start(
                    xs[:, cch * 4:(cch + 1) * 4, :], xsr[:, cch * 4:(cch + 1) * 4, :]
                )
            qT = persist.tile([P, HT, SSH], BF16)
            nc.sync.dma_start(qT[:], qTi.rearrange("p (a s) -> p a s", a=HT))

            kb0 = persist.tile([P, HT, SSH], BF16)
            nc.sync.dma_start(
                kb0[:], kT_all[0:P, :].rearrange("p (a s) -> p a s", a=HT)
            )

            # ======== MLP, sequence-sharded, full weights ========
            # Two expert-halves of 4096 so gelu(fc1) [e, s] needs only a
            # half-size buffer; the second fc2 pass accumulates via DVE add.
            scope_mlp = nc.named_scope("mlp"); scope_mlp.__enter__()
            h_sb = persist.tile([P, ST, HID], F32)  # local MLP output (f32)
            EHALF = ET // 2
            with tc.tile_pool(name="w1B", bufs=2) as w1B, \
                 tc.tile_pool(name="w2B", bufs=2) as w2B, \
                 tc.tile_pool(name="gB", bufs=1) as gB, \
                 tc.tile_pool(name="psB", bufs=3, space="PSUM") as psB, \
                 tc.tile_pool(name="psB2", bufs=1, space="PSUM") as psB2:
                for half in range(2):
                    g1 = gB.tile([P, EHALF, SSH], BF16, name="g1")
                    for eg in range(8):
                        ego = half * 8 + eg
                        w1g = w1B.tile([P, HT, 512], BF16, name="w1g")
                        w1sl = _wslice(w1, ego, HT)
                        if half == 0 and eg == 0:
                            for cch in range(4):
                                nc.sync.dma_start(
                                    w1g[:, cch * 4:(cch + 1) * 4, :],
                                    w1sl[:, cch * 4:(cch + 1) * 4, :],
                                )
                        else:
                            nc.sync.dma_start(w1g[:], w1sl)
                        for m in range(4):
                            pt = psB.tile([P, SSH], F32, name="pB1")
                            for k in range(HT):
                                nc.tensor.matmul(
                                    pt[:], w1g[:, k, m * P:(m + 1) * P], xs[:, k, :],
                                    start=(k == 0), stop=(k == HT - 1),
                                )
                            nc.scalar.activation(
                                g1[:, eg * 4 + m, :], pt[:], AF.Gelu,
                                bias=b1_s[:, ego * 4 + m:ego * 4 + m + 1],
                            )
                    for n in range(4):
                        pts = [psB2.tile([P, 512], F32, name=f"pB2{m}")
                               for m in range(ST)]
                        for qtr in range(2):
                            base = (n * 16 + half * 8 + qtr * 4) * 2048
                            w2g = w2B.tile([P, 16, 512], BF16, name="w2g")
                            nc.sync.dma_start(
                                w2g[:],
                                w2[:, base:base + 8192].rearrange(
                                    "p (a s) -> p a s", a=16
                                ),
                            )
                            for m in range(ST):
                                for kk in range(16):
                                    k = qtr * 16 + kk
                                    nc.tensor.matmul(
                                        pts[m][:], g1[:, k, m * P:(m + 1) * P],
                                        w2g[:, kk, :],
                                        start=(k == 0), stop=(k == EHALF - 1),
                                    )
                        for m in range(ST):
                            if half == 0:
                                nc.vector.tensor_copy(
                                    h_sb[:, m, n * 512:(n + 1) * 512], pts[m][:]
                                )
                            else:
                                nc.vector.tensor_add(
                                    h_sb[:, m, n * 512:(n + 1) * 512],
                                    h_sb[:, m, n * 512:(n + 1) * 512], pts[m][:],
                                )
            scope_mlp.__exit__(None, None, None)

            # ================= attention =================
            scope_att = nc.named_scope("attn"); scope_att.__enter__()
            attT = persist.tile([P, HT, SSH], BF16)     # (E @ v).T
            recip = persist.tile([P, ST], F32)

            kC_cm = tc.tile_pool(name="kC", bufs=2)
            kC = kC_cm.__enter__()
            ots = {}
            with tc.tile_pool(name="aC", bufs=1) as aC, \
                 tc.tile_pool(name="vC", bufs=2) as vC, \
                 tc.tile_pool(name="psC", bufs=2, space="PSUM") as psC, \
                 tc.tile_pool(name="psR", bufs=1, space="PSUM") as psR, \
                 tc.tile_pool(name="psV", bufs=1, space="PSUM") as psV:
                attnT = aC.tile([P, GT, SSH], BF16)   # exp(scores).T (unnormalized)
                for mb in range(NCORES):
                    if mb == 0:
                        kb = kb0
                    else:
                        kb = kC.tile([P, HT, SSH], BF16, name="kb")
                        nc.sync.dma_start(
                            kb[:],
                            kT_all[mb * P:(mb + 1) * P, :].rearrange(
                                "p (a s) -> p a s", a=HT
                            ),
                        )
                    for mm in range(4):
                        pt = psC.tile([P, SSH], F32, name="pC")
                        for k in range(HT):
                            nc.tensor.matmul(
                                pt[:], kb[:, k, mm * P:(mm + 1) * P], qT[:, k, :],
                                start=(k == 0), stop=(k == HT - 1),
                            )
                        nc.scalar.activation(
                            attnT[:, mb * 4 + mm, :], pt[:], AF.Exp, scale=EXPSCALE
                        )

                # prefetch the first o-projection weight block through the
                # same pool slots the kb tiles used
                ot0 = kC.tile([P, HT, 512], BF16, name="kb")
                nc.sync.dma_start(ot0[:], _wslice(wo, 0, HT))
                ots[0] = ot0

                # E @ v, transposed: attT[h, s_local]
                for g in range(4):
                    pts = [psV.tile([P, SSH], F32, name=f"pV{m}") for m in range(4)]
                    for q4 in range(2):
                        vt = vC.tile([P, 16, 512], BF16, name="vt")
                        for j in range(4):
                            rb = q4 * 4 + j
                            nc.sync.dma_start(
                                vt[:, j * 4:(j + 1) * 4, :],
                                v_all[rb * P:(rb + 1) * P, :].rearrange(
                                    "p (a s) -> p a s", a=ST
                                )[:, :, g * 512:(g + 1) * 512],
                            )
                        for m in range(4):
                            for kk in range(16):
                                k = q4 * 16 + kk
                                nc.tensor.matmul(
                                    pts[m][:], vt[:, kk, m * P:(m + 1) * P],
                                    attnT[:, k, :],
                                    start=(k == 0), stop=(k == GT - 1),
                                )
                    for m in range(4):
                        nc.vector.tensor_copy(attT[:, g * 4 + m, :], pts[m][:])

                # softmax row sums via ones-matmuls
                prs = psR.tile([P, ST], F32)
                for k in range(GT):
                    for m2 in range(ST):
                        nc.tensor.matmul(
                            prs[:, m2:m2 + 1], attnT[:, k, m2 * P:(m2 + 1) * P],
                            ones[:], start=(k == 0), stop=(k == GT - 1),
                        )
                nc.vector.reciprocal(recip[:], prs[:])
            scope_att.__exit__(None, None, None)

            # ============ output projection + combine ============
            scope_o = nc.named_scope("oproj"); scope_o.__enter__()
            with tc.tile_pool(name="evD", bufs=4) as evD, \
                 tc.tile_pool(name="psD", bufs=3, space="PSUM") as psD:
                for n in range(4):
                    if n in ots:
                        ot = ots[n]
                    else:
                        ot = kC.tile([P, HT, 512], BF16, name="kb")
                        nc.sync.dma_start(ot[:], _wslice(wo, n, HT))
                    for m in range(ST):
                        pt = psD.tile([P, 512], F32, name="pD")
                        for k in range(HT):
                            nc.tensor.matmul(
                                pt[:], attT[:, k, m * P:(m + 1) * P], ot[:, k, :],
                                start=(k == 0), stop=(k == HT - 1),
                            )
                        ev = evD.tile([P, 512], F32, name="evD")
                        nc.vector.tensor_scalar_mul(ev[:], pt[:], recip[:, m:m + 1])
                        nc.vector.tensor_add(
                            ev[:], ev[:], h_sb[:, m, n * 512:(n + 1) * 512]
                        )
                        nc.sync.dma_start(
                            out[m * P:(m + 1) * P, n * 512:(n + 1) * 512], ev[:]
                        )
            scope_o.__exit__(None, None, None)
            kC_cm.__exit__(None, None, None)

    nc.compile()
    return nc


def _get_ncs():
    if "qkv" not in _CACHE:
        _CACHE["qkv"] = _build_qkv()
        _CACHE["main"] = _build_main()
    return _CACHE["qkv"], _CACHE["main"]


def _swizzle(wT, nb):
    """[K, N] (contraction-major) -> SBUF image [128, (N/nb) * (K/128) * nb]:
    out[p, b, a, s] = wT[a*128 + p, b*nb + s], flattened over (b, a, s)."""
    K, N = wT.shape
    kt, npb = K // P, N // nb
    return np.ascontiguousarray(
        wT.reshape(kt, P, npb, nb).transpose(1, 2, 0, 3).reshape(P, npb * kt * nb)
    )


def _prep(x, fc1_w, fc1_b, fc2_w, fc2_b, q_w, q_b, k_w, k_b, v_w, v_b, o_w, o_b):
    f32 = np.float32
    xT_bf = np.ascontiguousarray(np.asarray(x, f32).T).astype(BF_NP)
    wq_t = _swizzle(np.asarray(q_w, f32).T.astype(BF_NP), 512)
    wk_t = _swizzle(np.asarray(k_w, f32).T.astype(BF_NP), 512)
    wv_t = _swizzle(np.asarray(v_w, f32).T.astype(BF_NP), 512)
    wo_t = _swizzle(np.asarray(o_w, f32).T.astype(BF_NP), 512)
    w1_t = _swizzle(np.asarray(fc1_w, f32).T.astype(BF_NP), 512)
    w2T = np.asarray(fc2_w, f32).T.astype(BF_NP)                   # [EXP, HID]
    # fc2 stream layout [p, n(4), kq(16), kk(4), s(512)]:
    # element = w2T[(kq*4+kk)*128 + p, n*512 + s]
    w2_t = np.ascontiguousarray(
        w2T.reshape(16, 4, P, 4, 512).transpose(2, 3, 0, 1, 4).reshape(P, -1)
    )
    qb2 = np.ascontiguousarray(np.asarray(q_b, f32).reshape(HT, P).T)
    kb2 = np.ascontiguousarray(np.asarray(k_b, f32).reshape(HT, P).T)
    b12 = np.ascontiguousarray(np.asarray(fc1_b, f32).reshape(ET, P).T)

    xsh_imgs = []
    for c in range(NCORES):
        xc = np.ascontiguousarray(xT_bf[:, c * SSH:(c + 1) * SSH])
        xsh_imgs.append(np.ascontiguousarray(
            xc.reshape(HT, P, SSH).transpose(1, 0, 2).reshape(P, -1)
        ))
    host_add = (
        np.asarray(fc2_b, f32)
        + np.asarray(o_b, f32)
        + np.asarray(o_w, f32) @ np.asarray(v_b, f32)
    )
    return {
        "xsh": xsh_imgs, "wq": wq_t, "wk": wk_t, "wv": wv_t, "wo": wo_t,
        "w1": w1_t, "w2": w2_t, "qb2": qb2, "kb2": kb2, "b12": b12,
        "host_add": host_add,
    }


def run(trace=False, tmpdir=None, **inputs):
    nc1, nc2 = _get_ncs()
    pp = _prep(**inputs)
    if tmpdir:
        os.makedirs(tmpdir + "/l1", exist_ok=True)
        os.makedirs(tmpdir + "/l2", exist_ok=True)
    in1 = [{
        "xsh": pp["xsh"][c], "wq": pp["wq"], "wk": pp["wk"], "wv": pp["wv"],
        "qb2": pp["qb2"], "kb2": pp["kb2"],
    } for c in range(NCORES)]
    res1 = run_bass_kernel_spmd(
        nc1, in1, core_ids=list(range(NCORES)), trace=trace,
        tmpdir=(tmpdir + "/l1") if tmpdir else None,
    )
    kT_all = np.concatenate([res1.results[c]["kT_o"] for c in range(NCORES)], axis=0)
    v_all = np.concatenate([res1.results[c]["v_o"] for c in range(NCORES)], axis=0)

    in2 = [{
        "xsh": pp["xsh"][c], "qTi": res1.results[c]["qT_o"],
        "kT_all": kT_all, "v_all": v_all,
        "wo": pp["wo"], "w1": pp["w1"], "w2": pp["w2"], "b12": pp["b12"],
    } for c in range(NCORES)]
    res2 = run_bass_kernel_spmd(
        nc2, in2, core_ids=list(range(NCORES)), trace=trace,
        tmpdir=(tmpdir + "/l2") if tmpdir else None,
    )
    outp = np.concatenate(
        [res2.results[c]["out"] for c in range(NCORES)], axis=0
    ) + pp["host_add"][None, :]
    return outp.astype(np.float32), (res1, res2)


def kernel(**inputs):
    outp, _ = run(trace=False, **inputs)
    return outp


# revision 30
# speedup vs baseline: 1.0022x; 1.0022x over previous
"""Trainium2 Bass kernel for nn_DistributedExpert (dense transformer expert).

Computes, for x [4096, 2048]:
    h   = gelu(x @ fc1_w.T + fc1_b) @ fc2_w.T + fc2_b          (MLP branch)
    q/k/v = x @ {q,k,v}_w.T + b
    attn  = softmax(q @ k.T / sqrt(2048))
    out = (attn @ v) @ o_w.T + o_b + h

Distribution over 8 NeuronCores — everything is sequence-sharded (each core
owns 512 rows of x and of the output). Two collective-free launches:

  Launch 1: each core computes q/k/v (transposed layouts) for its rows.
  Host:     gathers the k/v shards (this replaces an on-device AllGather —
            measured: having ANY collective in the NEFF slows every matmul
            by ~21%, 216 -> 263 ns, so the gather is done on the host).
  Launch 2: MLP (full weights streamed, no expert sharding) + attention +
            output projection + combine.

  - Softmax uses the no-max-subtraction form (scores are O(1)); row sums are
    built with ones-matmuls on the transposed attention layout and the
    normalization is applied after the output projection.
  - Biases with an easy per-partition broadcast (q_b, k_b, fc1_b) are added
    on-device; v_b/o_b/fc2_b contributions are mathematically exact constant
    row-vectors, added on the host.

Matmuls run in bf16 (fp32 PSUM accumulation). All streamed weights are
pre-swizzled on the host into their exact SBUF image ([128 partitions x
contiguous free bytes]) so every weight DMA is a full-bandwidth linear copy.
"""

import os
import sys

sys.path.insert(0, "/opt/trn_rl_repo")

import numpy as np
import ml_dtypes

import concourse.bass as bass
import concourse.mybir as mybir
import concourse.tile as tile
from concourse import bacc
from concourse.bass_utils import run_bass_kernel_spmd

SEQ = 4096
HID = 2048
EXP = 8192
NCORES = 8
SSH = SEQ // NCORES   # 512 sequence rows per core
P = 128

HT = HID // P        # 16 hidden tiles
ET = EXP // P        # 64 expert tiles
ST = SSH // P        # 4 local-seq tiles
GT = SEQ // P        # 32 global-seq tiles

F32 = mybir.dt.float32
BF16 = mybir.dt.bfloat16
AF = mybir.ActivationFunctionType
BF_NP = ml_dtypes.bfloat16

_CACHE = {}


def _wslice(w_ap, blk, ntiles):
    # [128, ntiles, 512] SBUF-image slice for output-block `blk`
    sz = ntiles * 512
    return w_ap[:, blk * sz:(blk + 1) * sz].rearrange("p (a s) -> p a s", a=ntiles)


def _build_qkv():
    nc = bacc.Bacc("TRN2", target_bir_lowering=False, debug=False, num_devices=NCORES)
    xsh = nc.dram_tensor("xsh", [P, HT * SSH], BF16, kind="ExternalInput").ap()
    wq = nc.dram_tensor("wq", [P, 4 * HT * 512], BF16, kind="ExternalInput").ap()
    wk = nc.dram_tensor("wk", [P, 4 * HT * 512], BF16, kind="ExternalInput").ap()
    wv = nc.dram_tensor("wv", [P, 4 * HT * 512], BF16, kind="ExternalInput").ap()
    qb2 = nc.dram_tensor("qb2", [P, HT], F32, kind="ExternalInput").ap()
    kb2 = nc.dram_tensor("kb2", [P, HT], F32, kind="ExternalInput").ap()
    qT_o = nc.dram_tensor("qT_o", [P, HT * SSH], BF16, kind="ExternalOutput").ap()
    kT_o = nc.dram_tensor("kT_o", [P, HT * SSH], BF16, kind="ExternalOutput").ap()
    v_o = nc.dram_tensor("v_o", [P, ST * HID], BF16, kind="ExternalOutput").ap()

    with tile.TileContext(nc) as tc:
        with tc.tile_pool(name="const", bufs=1) as constp, \
             tc.tile_pool(name="st", bufs=1) as st, \
             tc.tile_pool(name="ws", bufs=3) as ws, \
             tc.tile_pool(name="ps", bufs=3, space="PSUM") as ps:
            qb_s = constp.tile([P, HT], F32)
            kb_s = constp.tile([P, HT], F32)
            nc.sync.dma_start(qb_s[:], qb2[:])
            nc.sync.dma_start(kb_s[:], kb2[:])
            xs = st.tile([P, HT, SSH], BF16)
            xsr = xsh.rearrange("p (a s) -> p a s", a=HT)
            for cch in range(4):
                nc.sync.dma_start(
                    xs[:, cch * 4:(cch + 1) * 4, :], xsr[:, cch * 4:(cch + 1) * 4, :]
                )
            qT = st.tile([P, HT, SSH], BF16)
            kT = st.tile([P, HT, SSH], BF16)
            v_s = st.tile([P, ST, HID], BF16)

            for dst, dst_o, w_ap, bias in (
                (qT, qT_o, wq, qb_s), (kT, kT_o, wk, kb_s)
            ):
                for g in range(4):
                    wt = ws.tile([P, HT, 512], BF16, name="wtile")
                    wsl = _wslice(w_ap, g, HT)
                    for cch in range(4):
                        nc.sync.dma_start(
                            wt[:, cch * 4:(cch + 1) * 4, :],
                            wsl[:, cch * 4:(cch + 1) * 4, :],
                        )
                    for m in range(4):
                        pt = ps.tile([P, SSH], F32, name="pA")
                        for k in range(HT):
                            nc.tensor.matmul(
                                pt[:], wt[:, k, m * P:(m + 1) * P], xs[:, k, :],
                                start=(k == 0), stop=(k == HT - 1),
                            )
                        nc.scalar.activation(
                            dst[:, g * 4 + m, :], pt[:], AF.Identity,
                            bias=bias[:, g * 4 + m:g * 4 + m + 1],
                        )
                    # stream this group's 4 tiles out while the next computes
                    nc.sync.dma_start(
                        _wslice(dst_o, g, 4), dst[:, g * 4:(g + 1) * 4, :]
                    )
            # v in natural layout [s_local, hid]  (v_b folded on host)
            for n in range(4):
                wt = ws.tile([P, HT, 512], BF16, name="wtile")
                nc.sync.dma_start(wt[:], _wslice(wv, n, HT))
                for m in range(ST):
                    pt = ps.tile([P, SSH], F32, name="pA")
                    for k in range(HT):
                        nc.tensor.matmul(
                            pt[:], xs[:, k, m * P:(m + 1) * P], wt[:, k, :],
                            start=(k == 0), stop=(k == HT - 1),
                        )
                    nc.vector.tensor_copy(v_s[:, m, n * 512:(n + 1) * 512], pt[:])
                nc.sync.dma_start(
                    v_o.rearrange("p (a s) -> p a s", a=ST)[:, :, n * 512:(n + 1) * 512],
                    v_s[:, :, n * 512:(n + 1) * 512],
                )
    nc.compile()
    return nc


def _build_main():
    nc = bacc.Bacc("TRN2", target_bir_lowering=False, debug=False, num_devices=NCORES)
    EXPSCALE = 1.0 / float(np.sqrt(np.float32(HID)))

    xsh = nc.dram_tensor("xsh", [P, HT * SSH], BF16, kind="ExternalInput").ap()
    qTi = nc.dram_tensor("qTi", [P, HT * SSH], BF16, kind="ExternalInput").ap()
    kT_all = nc.dram_tensor("kT_all", [NCORES * P, HT * SSH], BF16,
                            kind="ExternalInput").ap()
    v_all = nc.dram_tensor("v_all", [NCORES * P, ST * HID], BF16,
                           kind="ExternalInput").ap()
    wo = nc.dram_tensor("wo", [P, 4 * HT * 512], BF16, kind="ExternalInput").ap()
    w1 = nc.dram_tensor("w1", [P, 16 * HT * 512], BF16, kind="ExternalInput").ap()
    w2 = nc.dram_tensor("w2", [P, 4 * ET * 512], BF16, kind="ExternalInput").ap()
    b12 = nc.dram_tensor("b12", [P, ET], F32, kind="ExternalInput").ap()
    out = nc.dram_tensor("out", [SSH, HID], F32, kind="ExternalOutput").ap()

    with tile.TileContext(nc) as tc:
        with tc.tile_pool(name="const", bufs=1) as constp, \
             tc.tile_pool(name="persist", bufs=1) as persist:
            ones = constp.tile([P, 1], BF16)
            nc.vector.memset(ones[:], 1.0)
            b1_s = constp.tile([P, ET], F32)
            nc.sync.dma_start(b1_s[:], b12[:])

            xs = persist.tile([P, HT, SSH], BF16)
            xsr = xsh.rearrange("p (a s) -> p a s", a=HT)
            for cch in range(4):
                nc.sync.dma_start(
                    xs[:, cch * 4:(cch + 1) * 4, :], xsr[:, cch * 4:(cch + 1) * 4, :]
                )
            qT = persist.tile([P, HT, SSH], BF16)
            nc.sync.dma_start(qT[:], qTi.rearrange("p (a s) -> p a s", a=HT))

            kb0 = persist.tile([P, HT, SSH], BF16)
            nc.sync.dma_start(
                kb0[:], kT_all[0:P, :].rearrange("p (a s) -> p a s", a=HT)
            )

            # ======== MLP, sequence-sharded, full weights ========
            # Two expert-halves of 4096 so gelu(fc1) [e, s] needs only a
            # half-size buffer; the second fc2 pass accumulates via DVE add.
            scope_mlp = nc.named_scope("mlp"); scope_mlp.__enter__()
            h_sb = persist.tile([P, ST, HID], F32)  # local MLP output (f32)
            EHALF = ET // 2
            with tc.tile_pool(name="w1B", bufs=2) as w1B, \
                 tc.tile_pool(name="w2B", bufs=2) as w2B, \
                 tc.tile_pool(name="gB", bufs=1) as gB, \
                 tc.tile_pool(name="psB", bufs=3, space="PSUM") as psB, \
                 tc.tile_pool(name="psB2", bufs=1, space="PSUM") as psB2:
                for half in range(2):
                    g1 = gB.tile([P, EHALF, SSH], BF16, name="g1")
                    for eg in range(8):
                        ego = half * 8 + eg
                        w1g = w1B.tile([P, HT, 512], BF16, name="w1g")
                        w1sl = _wslice(w1, ego, HT)
                        if half == 0 and eg == 0:
                            for cch in range(4):
                                nc.sync.dma_start(
                                    w1g[:, cch * 4:(cch + 1) * 4, :],
                                    w1sl[:, cch * 4:(cch + 1) * 4, :],
                                )
                        else:
                            nc.sync.dma_start(w1g[:], w1sl)
                        for m in range(4):
                            pt = psB.tile([P, SSH], F32, name="pB1")
                            for k in range(HT):
                                nc.tensor.matmul(
                                    pt[:], w1g[:, k, m * P:(m + 1) * P], xs[:, k, :],
                                    start=(k == 0), stop=(k == HT - 1),
                                )
                            nc.scalar.activation(
                                g1[:, eg * 4 + m, :], pt[:], AF.Gelu,
                                bias=b1_s[:, ego * 4 + m:ego * 4 + m + 1],
                            )
                    for n in range(4):
                        pts = [psB2.tile([P, 512], F32, name=f"pB2{m}")
                               for m in range(ST)]
                        for qtr in range(2):
                            base = (n * 16 + half * 8 + qtr * 4) * 2048
                            w2g = w2B.tile([P, 16, 512], BF16, name="w2g")
                            nc.sync.dma_start(
                                w2g[:],
                                w2[:, base:base + 8192].rearrange(
                                    "p (a s) -> p a s", a=16
                                ),
                            )
                            for m in range(ST):
                                for kk in range(16):
                                    k = qtr * 16 + kk
                                    nc.tensor.matmul(
                                        pts[m][:], g1[:, k, m * P:(m + 1) * P],
                                        w2g[:, kk, :],
                                        start=(k == 0), stop=(k == EHALF - 1),
                                    )
                        for m in range(ST):
                            if half == 0:
                                nc.vector.tensor_copy(
                                    h_sb[:, m, n * 512:(n + 1) * 512], pts[m][:]
                                )
                            else:
                                nc.vector.tensor_add(
                                    h_sb[:, m, n * 512:(n + 1) * 512],
                                    h_sb[:, m, n * 512:(n + 1) * 512], pts[m][:],
                                )
            scope_mlp.__exit__(None, None, None)

            # ================= attention =================
            scope_att = nc.named_scope("attn"); scope_att.__enter__()
            attT = persist.tile([P, HT, SSH], BF16)     # (E @ v).T
            recip = persist.tile([P, ST], F32)

            kC_cm = tc.tile_pool(name="kC", bufs=2)
            kC = kC_cm.__enter__()
            ots = {}
            with tc.tile_pool(name="aC", bufs=1) as aC, \
                 tc.tile_pool(name="vC", bufs=2) as vC, \
                 tc.tile_pool(name="psC", bufs=2, space="PSUM") as psC, \
                 tc.tile_pool(name="psR", bufs=1, space="PSUM") as psR, \
                 tc.tile_pool(name="psV", bufs=1, space="PSUM") as psV:
                attnT = aC.tile([P, GT, SSH], BF16)   # exp(scores).T (unnormalized)
                for mb in range(NCORES):
                    if mb == 0:
                        kb = kb0
                    else:
                        kb = kC.tile([P, HT, SSH], BF16, name="kb")
                        nc.sync.dma_start(
                            kb[:],
                            kT_all[mb * P:(mb + 1) * P, :].rearrange(
                                "p (a s) -> p a s", a=HT
                            ),
                        )
                    for mm in range(4):
                        pt = psC.tile([P, SSH], F32, name="pC")
                        for k in range(HT):
                            nc.tensor.matmul(
                                pt[:], kb[:, k, mm * P:(mm + 1) * P], qT[:, k, :],
                                start=(k == 0), stop=(k == HT - 1),
                            )
                        nc.scalar.activation(
                            attnT[:, mb * 4 + mm, :], pt[:], AF.Exp, scale=EXPSCALE
                        )

                # prefetch the first o-projection weight block through the
                # same pool slots the kb tiles used
                ot0 = kC.tile([P, HT, 512], BF16, name="kb")
                nc.sync.dma_start(ot0[:], _wslice(wo, 0, HT))
                ots[0] = ot0

                # E @ v, transposed: attT[h, s_local]
                for g in range(4):
                    pts = [psV.tile([P, SSH], F32, name=f"pV{m}") for m in range(4)]
                    for q4 in range(2):
                        vt = vC.tile([P, 16, 512], BF16, name="vt")
                        for j in range(4):
                            rb = q4 * 4 + j
                            nc.sync.dma_start(
                                vt[:, j * 4:(j + 1) * 4, :],
                                v_all[rb * P:(rb + 1) * P, :].rearrange(
                                    "p (a s) -> p a s", a=ST
                                )[:, :, g * 512:(g + 1) * 512],
                            )
                        for m in range(4):
                            for kk in range(16):
                                k = q4 * 16 + kk
                                nc.tensor.matmul(
                                    pts[m][:], vt[:, kk, m * P:(m + 1) * P],
                                    attnT[:, k, :],
                                    start=(k == 0), stop=(k == GT - 1),
                                )
                    for m in range(4):
                        nc.vector.tensor_copy(attT[:, g * 4 + m, :], pts[m][:])

                # softmax row sums via ones-matmuls
                prs = psR.tile([P, ST], F32)
                for k in range(GT):
                    for m2 in range(ST):
                        nc.tensor.matmul(
                            prs[:, m2:m2 + 1], attnT[:, k, m2 * P:(m2 + 1) * P],
                            ones[:], start=(k == 0), stop=(k == GT - 1),
                        )
                nc.vector.reciprocal(recip[:], prs[:])
            scope_att.__exit__(None, None, None)

            # ============ output projection + combine ============
            scope_o = nc.named_scope("oproj"); scope_o.__enter__()
            with tc.tile_pool(name="evD", bufs=4) as evD, \
                 tc.tile_pool(name="psD", bufs=3, space="PSUM") as psD:
                for n in range(4):
                    if n in ots:
                        ot = ots[n]
                    else:
                        ot = kC.tile([P, HT, 512], BF16, name="kb")
                        nc.sync.dma_start(ot[:], _wslice(wo, n, HT))
                    for m in range(ST):
                        pt = psD.tile([P, 512], F32, name="pD")
                        for k in range(HT):
                            nc.tensor.matmul(
                                pt[:], attT[:, k, m * P:(m + 1) * P], ot[:, k, :],
                                start=(k == 0), stop=(k == HT - 1),
                            )
                        ev = evD.tile([P, 512], F32, name="evD")
                        nc.vector.tensor_scalar_mul(ev[:], pt[:], recip[:, m:m + 1])
                        nc.vector.tensor_add(
                            ev[:], ev[:], h_sb[:, m, n * 512:(n + 1) * 512]
                        )
                        nc.sync.dma_start(
                            out[m * P:(m + 1) * P, n * 512:(n + 1) * 512], ev[:]
                        )
            scope_o.__exit__(None, None, None)
            kC_cm.__exit__(None, None, None)

    nc.compile()
    return nc


def _get_ncs():
    if "qkv" not in _CACHE:
        _CACHE["qkv"] = _build_qkv()
        _CACHE["main"] = _build_main()
    return _CACHE["qkv"], _CACHE["main"]


def _swizzle(wT, nb):
    """[K, N] (contraction-major) -> SBUF image [128, (N/nb) * (K/128) * nb]:
    out[p, b, a, s] = wT[a*128 + p, b*nb + s], flattened over (b, a, s)."""
    K, N = wT.shape
    kt, npb = K // P, N // nb
    return np.ascontiguousarray(
        wT.reshape(kt, P, npb, nb).transpose(1, 2, 0, 3).reshape(P, npb * kt * nb)
    )


def _prep(x, fc1_w, fc1_b, fc2_w, fc2_b, q_w, q_b, k_w, k_b, v_w, v_b, o_w, o_b):
    f32 = np.float32
    xT_bf = np.ascontiguousarray(np.asarray(x, f32).T).astype(BF_NP)
    wq_t = _swizzle(np.asarray(q_w, f32).T.astype(BF_NP), 512)
    wk_t = _swizzle(np.asarray(k_w, f32).T.astype(BF_NP), 512)
    wv_t = _swizzle(np.asarray(v_w, f32).T.astype(BF_NP), 512)
    wo_t = _swizzle(np.asarray(o_w, f32).T.astype(BF_NP), 512)
    w1_t = _swizzle(np.asarray(fc1_w, f32).T.astype(BF_NP), 512)
    w2T = np.asarray(fc2_w, f32).T.astype(BF_NP)                   # [EXP, HID]
    # fc2 stream layout [p, n(4), kq(16), kk(4), s(512)]:
    # element = w2T[(kq*4+kk)*128 + p, n*512 + s]
    w2_t = np.ascontiguousarray(
        w2T.reshape(16, 4, P, 4, 512).transpose(2, 3, 0, 1, 4).reshape(P, -1)
    )
    qb2 = np.ascontiguousarray(np.asarray(q_b, f32).reshape(HT, P).T)
    kb2 = np.ascontiguousarray(np.asarray(k_b, f32).reshape(HT, P).T)
    b12 = np.ascontiguousarray(np.asarray(fc1_b, f32).reshape(ET, P).T)

    xsh_imgs = []
    for c in range(NCORES):
        xc = np.ascontiguousarray(xT_bf[:, c * SSH:(c + 1) * SSH])
        xsh_imgs.append(np.ascontiguousarray(
            xc.reshape(HT, P, SSH).transpose(1, 0, 2).reshape(P, -1)
        ))
    host_add = (
        np.asarray(fc2_b, f32)
        + np.asarray(o_b, f32)
        + np.asarray(o_w, f32) @ np.asarray(v_b, f32)
    )
    return {
        "xsh": xsh_imgs, "wq": wq_t, "wk": wk_t, "wv": wv_t, "wo": wo_t,
        "w1": w1_t, "w2": w2_t, "qb2": qb2, "kb2": kb2, "b12": b12,
        "host_add": host_add,
    }


def run(trace=False, tmpdir=None, **inputs):
    nc1, nc2 = _get_ncs()
    pp = _prep(**inputs)
    if tmpdir:
        os.makedirs(tmpdir + "/l1", exist_ok=True)
        os.makedirs(tmpdir + "/l2", exist_ok=True)
    in1 = [{
        "xsh": pp["xsh"][c], "wq": pp["wq"], "wk": pp["wk"], "wv": pp["wv"],
        "qb2": pp["qb2"], "kb2": pp["kb2"],
    } for c in range(NCORES)]
    res1 = run_bass_kernel_spmd(
        nc1, in1, core_ids=list(range(NCORES)), trace=trace,
        tmpdir=(tmpdir + "/l1") if tmpdir else None,
    )
    kT_all = np.concatenate([res1.results[c]["kT_o"] for c in range(NCORES)], axis=0)
    v_all = np.concatenate([res1.results[c]["v_o"] for c in range(NCORES)], axis=0)

    in2 = [{
        "xsh": pp["xsh"][c], "qTi": res1.results[c]["qT_o"],
        "kT_all": kT_all, "v_all": v_all,
        "wo": pp["wo"], "w1": pp["w1"], "w2": pp["w2"], "b12": pp["b12"],
    } for c in range(NCORES)]
    res2 = run_bass_kernel_spmd(
        nc2, in2, core_ids=list(range(NCORES)), trace=trace,
        tmpdir=(tmpdir + "/l2") if tmpdir else None,
    )
    outp = np.concatenate(
        [res2.results[c]["out"] for c in range(NCORES)], axis=0
    ) + pp["host_add"][None, :]
    return outp.astype(np.float32), (res1, res2)


def kernel(**inputs):
    outp, _ = run(trace=False, **inputs)
    return outp


# revision 31
# speedup vs baseline: 1.0033x; 1.0011x over previous
"""Trainium2 Bass kernel for nn_DistributedExpert (dense transformer expert).

Computes, for x [4096, 2048]:
    h   = gelu(x @ fc1_w.T + fc1_b) @ fc2_w.T + fc2_b          (MLP branch)
    q/k/v = x @ {q,k,v}_w.T + b
    attn  = softmax(q @ k.T / sqrt(2048))
    out = (attn @ v) @ o_w.T + o_b + h

Distribution over 8 NeuronCores — everything is sequence-sharded (each core
owns 512 rows of x and of the output). Two collective-free launches:

  Launch 1: each core computes q/k/v (transposed layouts) for its rows.
  Host:     gathers the k/v shards (this replaces an on-device AllGather —
            measured: having ANY collective in the NEFF slows every matmul
            by ~21%, 216 -> 263 ns, so the gather is done on the host).
  Launch 2: MLP (full weights streamed, no expert sharding) + attention +
            output projection + combine.

  - Softmax uses the no-max-subtraction form (scores are O(1)); row sums are
    built with ones-matmuls on the transposed attention layout and the
    normalization is applied after the output projection.
  - Biases with an easy per-partition broadcast (q_b, k_b, fc1_b) are added
    on-device; v_b/o_b/fc2_b contributions are mathematically exact constant
    row-vectors, added on the host.

Matmuls run in bf16 (fp32 PSUM accumulation). All streamed weights are
pre-swizzled on the host into their exact SBUF image ([128 partitions x
contiguous free bytes]) so every weight DMA is a full-bandwidth linear copy.
"""

import os
import sys

sys.path.insert(0, "/opt/trn_rl_repo")

import numpy as np
import ml_dtypes

import concourse.bass as bass
import concourse.mybir as mybir
import concourse.tile as tile
from concourse import bacc
from concourse.bass_utils import run_bass_kernel_spmd

SEQ = 4096
HID = 2048
EXP = 8192
NCORES = 8
SSH = SEQ // NCORES   # 512 sequence rows per core
P = 128

HT = HID // P        # 16 hidden tiles
ET = EXP // P        # 64 expert tiles
ST = SSH // P        # 4 local-seq tiles
GT = SEQ // P        # 32 global-seq tiles

F32 = mybir.dt.float32
BF16 = mybir.dt.bfloat16
AF = mybir.ActivationFunctionType
BF_NP = ml_dtypes.bfloat16

_CACHE = {}


def _wslice(w_ap, blk, ntiles):
    # [128, ntiles, 512] SBUF-image slice for output-block `blk`
    sz = ntiles * 512
    return w_ap[:, blk * sz:(blk + 1) * sz].rearrange("p (a s) -> p a s", a=ntiles)


def _build_qkv():
    nc = bacc.Bacc("TRN2", target_bir_lowering=False, debug=False, num_devices=NCORES)
    xsh = nc.dram_tensor("xsh", [P, HT * SSH], BF16, kind="ExternalInput").ap()
    wq = nc.dram_tensor("wq", [P, 4 * HT * 512], BF16, kind="ExternalInput").ap()
    wk = nc.dram_tensor("wk", [P, 4 * HT * 512], BF16, kind="ExternalInput").ap()
    wv = nc.dram_tensor("wv", [P, 4 * HT * 512], BF16, kind="ExternalInput").ap()
    qb2 = nc.dram_tensor("qb2", [P, HT], F32, kind="ExternalInput").ap()
    kb2 = nc.dram_tensor("kb2", [P, HT], F32, kind="ExternalInput").ap()
    qT_o = nc.dram_tensor("qT_o", [P, HT * SSH], BF16, kind="ExternalOutput").ap()
    kT_o = nc.dram_tensor("kT_o", [P, HT * SSH], BF16, kind="ExternalOutput").ap()
    v_o = nc.dram_tensor("v_o", [P, ST * HID], BF16, kind="ExternalOutput").ap()

    with tile.TileContext(nc) as tc:
        with tc.tile_pool(name="const", bufs=1) as constp, \
             tc.tile_pool(name="st", bufs=1) as st, \
             tc.tile_pool(name="ws", bufs=4) as ws, \
             tc.tile_pool(name="ps", bufs=4, space="PSUM") as ps:
            qb_s = constp.tile([P, HT], F32)
            kb_s = constp.tile([P, HT], F32)
            nc.sync.dma_start(qb_s[:], qb2[:])
            nc.sync.dma_start(kb_s[:], kb2[:])
            xs = st.tile([P, HT, SSH], BF16)
            xsr = xsh.rearrange("p (a s) -> p a s", a=HT)
            for cch in range(4):
                nc.sync.dma_start(
                    xs[:, cch * 4:(cch + 1) * 4, :], xsr[:, cch * 4:(cch + 1) * 4, :]
                )
            qT = st.tile([P, HT, SSH], BF16)
            kT = st.tile([P, HT, SSH], BF16)
            v_s = st.tile([P, ST, HID], BF16)

            for dst, dst_o, w_ap, bias in (
                (qT, qT_o, wq, qb_s), (kT, kT_o, wk, kb_s)
            ):
                for g in range(4):
                    wt = ws.tile([P, HT, 512], BF16, name="wtile")
                    wsl = _wslice(w_ap, g, HT)
                    for cch in range(4):
                        nc.sync.dma_start(
                            wt[:, cch * 4:(cch + 1) * 4, :],
                            wsl[:, cch * 4:(cch + 1) * 4, :],
                        )
                    for m in range(4):
                        pt = ps.tile([P, SSH], F32, name="pA")
                        for k in range(HT):
                            nc.tensor.matmul(
                                pt[:], wt[:, k, m * P:(m + 1) * P], xs[:, k, :],
                                start=(k == 0), stop=(k == HT - 1),
                            )
                        nc.scalar.activation(
                            dst[:, g * 4 + m, :], pt[:], AF.Identity,
                            bias=bias[:, g * 4 + m:g * 4 + m + 1],
                        )
                    # stream this group's 4 tiles out while the next computes
                    nc.sync.dma_start(
                        _wslice(dst_o, g, 4), dst[:, g * 4:(g + 1) * 4, :]
                    )
            # v in natural layout [s_local, hid]  (v_b folded on host)
            for n in range(4):
                wt = ws.tile([P, HT, 512], BF16, name="wtile")
                nc.sync.dma_start(wt[:], _wslice(wv, n, HT))
                for m in range(ST):
                    pt = ps.tile([P, SSH], F32, name="pA")
                    for k in range(HT):
                        nc.tensor.matmul(
                            pt[:], xs[:, k, m * P:(m + 1) * P], wt[:, k, :],
                            start=(k == 0), stop=(k == HT - 1),
                        )
                    nc.vector.tensor_copy(v_s[:, m, n * 512:(n + 1) * 512], pt[:])
                nc.sync.dma_start(
                    v_o.rearrange("p (a s) -> p a s", a=ST)[:, :, n * 512:(n + 1) * 512],
                    v_s[:, :, n * 512:(n + 1) * 512],
                )
    nc.compile()
    return nc


def _build_main():
    nc = bacc.Bacc("TRN2", target_bir_lowering=False, debug=False, num_devices=NCORES)
    EXPSCALE = 1.0 / float(np.sqrt(np.float32(HID)))

    xsh = nc.dram_tensor("xsh", [P, HT * SSH], BF16, kind="ExternalInput").ap()
    qTi = nc.dram_tensor("qTi", [P, HT * SSH], BF16, kind="ExternalInput").ap()
    kT_all = nc.dram_tensor("kT_all", [NCORES * P, HT * SSH], BF16,
                            kind="ExternalInput").ap()
    v_all = nc.dram_tensor("v_all", [NCORES * P, ST * HID], BF16,
                           kind="ExternalInput").ap()
    wo = nc.dram_tensor("wo", [P, 4 * HT * 512], BF16, kind="ExternalInput").ap()
    w1 = nc.dram_tensor("w1", [P, 16 * HT * 512], BF16, kind="ExternalInput").ap()
    w2 = nc.dram_tensor("w2", [P, 4 * ET * 512], BF16, kind="ExternalInput").ap()
    b12 = nc.dram_tensor("b12", [P, ET], F32, kind="ExternalInput").ap()
    out = nc.dram_tensor("out", [SSH, HID], F32, kind="ExternalOutput").ap()

    with tile.TileContext(nc) as tc:
        with tc.tile_pool(name="const", bufs=1) as constp, \
             tc.tile_pool(name="persist", bufs=1) as persist:
            ones = constp.tile([P, 1], BF16)
            nc.vector.memset(ones[:], 1.0)
            b1_s = constp.tile([P, ET], F32)
            nc.sync.dma_start(b1_s[:], b12[:])

            xs = persist.tile([P, HT, SSH], BF16)
            xsr = xsh.rearrange("p (a s) -> p a s", a=HT)
            for cch in range(4):
                nc.sync.dma_start(
                    xs[:, cch * 4:(cch + 1) * 4, :], xsr[:, cch * 4:(cch + 1) * 4, :]
                )
            qT = persist.tile([P, HT, SSH], BF16)
            nc.sync.dma_start(qT[:], qTi.rearrange("p (a s) -> p a s", a=HT))

            kb0 = persist.tile([P, HT, SSH], BF16)
            nc.sync.dma_start(
                kb0[:], kT_all[0:P, :].rearrange("p (a s) -> p a s", a=HT)
            )

            # ======== MLP, sequence-sharded, full weights ========
            # Two expert-halves of 4096 so gelu(fc1) [e, s] needs only a
            # half-size buffer; the second fc2 pass accumulates via DVE add.
            scope_mlp = nc.named_scope("mlp"); scope_mlp.__enter__()
            h_sb = persist.tile([P, ST, HID], F32)  # local MLP output (f32)
            EHALF = ET // 2
            with tc.tile_pool(name="w1B", bufs=2) as w1B, \
                 tc.tile_pool(name="w2B", bufs=2) as w2B, \
                 tc.tile_pool(name="gB", bufs=1) as gB, \
                 tc.tile_pool(name="psB", bufs=4, space="PSUM") as psB, \
                 tc.tile_pool(name="psB2", bufs=1, space="PSUM") as psB2:
                for half in range(2):
                    g1 = gB.tile([P, EHALF, SSH], BF16, name="g1")
                    for eg in range(8):
                        ego = half * 8 + eg
                        w1g = w1B.tile([P, HT, 512], BF16, name="w1g")
                        w1sl = _wslice(w1, ego, HT)
                        if half == 0 and eg == 0:
                            for cch in range(4):
                                nc.sync.dma_start(
                                    w1g[:, cch * 4:(cch + 1) * 4, :],
                                    w1sl[:, cch * 4:(cch + 1) * 4, :],
                                )
                        else:
                            nc.sync.dma_start(w1g[:], w1sl)
                        for m in range(4):
                            pt = psB.tile([P, SSH], F32, name="pB1")
                            for k in range(HT):
                                nc.tensor.matmul(
                                    pt[:], w1g[:, k, m * P:(m + 1) * P], xs[:, k, :],
                                    start=(k == 0), stop=(k == HT - 1),
                                )
                            nc.scalar.activation(
                                g1[:, eg * 4 + m, :], pt[:], AF.Gelu,
                                bias=b1_s[:, ego * 4 + m:ego * 4 + m + 1],
                            )
                    for n in range(4):
                        pts = [psB2.tile([P, 512], F32, name=f"pB2{m}")
                               for m in range(ST)]
                        for qtr in range(2):
                            base = (n * 16 + half * 8 + qtr * 4) * 2048
                            w2g = w2B.tile([P, 16, 512], BF16, name="w2g")
                            nc.sync.dma_start(
                                w2g[:],
                                w2[:, base:base + 8192].rearrange(
                                    "p (a s) -> p a s", a=16
                                ),
                            )
                            for m in range(ST):
                                for kk in range(16):
                                    k = qtr * 16 + kk
                                    nc.tensor.matmul(
                                        pts[m][:], g1[:, k, m * P:(m + 1) * P],
                                        w2g[:, kk, :],
                                        start=(k == 0), stop=(k == EHALF - 1),
                                    )
                        for m in range(ST):
                            if half == 0:
                                nc.vector.tensor_copy(
                                    h_sb[:, m, n * 512:(n + 1) * 512], pts[m][:]
                                )
                            else:
                                nc.vector.tensor_add(
                                    h_sb[:, m, n * 512:(n + 1) * 512],
                                    h_sb[:, m, n * 512:(n + 1) * 512], pts[m][:],
                                )
            scope_mlp.__exit__(None, None, None)

            # ================= attention =================
            scope_att = nc.named_scope("attn"); scope_att.__enter__()
            attT = persist.tile([P, HT, SSH], BF16)     # (E @ v).T
            recip = persist.tile([P, ST], F32)

            kC_cm = tc.tile_pool(name="kC", bufs=2)
            kC = kC_cm.__enter__()
            ots = {}
            with tc.tile_pool(name="aC", bufs=1) as aC, \
                 tc.tile_pool(name="vC", bufs=2) as vC, \
                 tc.tile_pool(name="psC", bufs=3, space="PSUM") as psC, \
                 tc.tile_pool(name="psR", bufs=1, space="PSUM") as psR, \
                 tc.tile_pool(name="psV", bufs=1, space="PSUM") as psV:
                attnT = aC.tile([P, GT, SSH], BF16)   # exp(scores).T (unnormalized)
                for mb in range(NCORES):
                    if mb == 0:
                        kb = kb0
                    else:
                        kb = kC.tile([P, HT, SSH], BF16, name="kb")
                        nc.sync.dma_start(
                            kb[:],
                            kT_all[mb * P:(mb + 1) * P, :].rearrange(
                                "p (a s) -> p a s", a=HT
                            ),
                        )
                    for mm in range(4):
                        pt = psC.tile([P, SSH], F32, name="pC")
                        for k in range(HT):
                            nc.tensor.matmul(
                                pt[:], kb[:, k, mm * P:(mm + 1) * P], qT[:, k, :],
                                start=(k == 0), stop=(k == HT - 1),
                            )
                        nc.scalar.activation(
                            attnT[:, mb * 4 + mm, :], pt[:], AF.Exp, scale=EXPSCALE
                        )

                # prefetch the first o-projection weight block through the
                # same pool slots the kb tiles used
                ot0 = kC.tile([P, HT, 512], BF16, name="kb")
                nc.sync.dma_start(ot0[:], _wslice(wo, 0, HT))
                ots[0] = ot0

                # E @ v, transposed: attT[h, s_local]
                for g in range(4):
                    pts = [psV.tile([P, SSH], F32, name=f"pV{m}") for m in range(4)]
                    for q4 in range(2):
                        vt = vC.tile([P, 16, 512], BF16, name="vt")
                        for j in range(4):
                            rb = q4 * 4 + j
                            nc.sync.dma_start(
                                vt[:, j * 4:(j + 1) * 4, :],
                                v_all[rb * P:(rb + 1) * P, :].rearrange(
                                    "p (a s) -> p a s", a=ST
                                )[:, :, g * 512:(g + 1) * 512],
                            )
                        for m in range(4):
                            for kk in range(16):
                                k = q4 * 16 + kk
                                nc.tensor.matmul(
                                    pts[m][:], vt[:, kk, m * P:(m + 1) * P],
                                    attnT[:, k, :],
                                    start=(k == 0), stop=(k == GT - 1),
                                )
                    for m in range(4):
                        nc.vector.tensor_copy(attT[:, g * 4 + m, :], pts[m][:])

                # softmax row sums via ones-matmuls
                prs = psR.tile([P, ST], F32)
                for k in range(GT):
                    for m2 in range(ST):
                        nc.tensor.matmul(
                            prs[:, m2:m2 + 1], attnT[:, k, m2 * P:(m2 + 1) * P],
                            ones[:], start=(k == 0), stop=(k == GT - 1),
                        )
                nc.vector.reciprocal(recip[:], prs[:])
            scope_att.__exit__(None, None, None)

            # ============ output projection + combine ============
            scope_o = nc.named_scope("oproj"); scope_o.__enter__()
            with tc.tile_pool(name="evD", bufs=4) as evD, \
                 tc.tile_pool(name="psD", bufs=4, space="PSUM") as psD:
                for n in range(4):
                    if n in ots:
                        ot = ots[n]
                    else:
                        ot = kC.tile([P, HT, 512], BF16, name="kb")
                        nc.sync.dma_start(ot[:], _wslice(wo, n, HT))
                    for m in range(ST):
                        pt = psD.tile([P, 512], F32, name="pD")
                        for k in range(HT):
                            nc.tensor.matmul(
                                pt[:], attT[:, k, m * P:(m + 1) * P], ot[:, k, :],
                                start=(k == 0), stop=(k == HT - 1),
                            )
                        ev = evD.tile([P, 512], F32, name="evD")
                        nc.vector.tensor_scalar_mul(ev[:], pt[:], recip[:, m:m + 1])
                        nc.vector.tensor_add(
                            ev[:], ev[:], h_sb[:, m, n * 512:(n + 1) * 512]
                        )
                        nc.sync.dma_start(
                            out[m * P:(m + 1) * P, n * 512:(n + 1) * 512], ev[:]
                        )
            scope_o.__exit__(None, None, None)
            kC_cm.__exit__(None, None, None)

    nc.compile()
    return nc


def _get_ncs():
    if "qkv" not in _CACHE:
        _CACHE["qkv"] = _build_qkv()
        _CACHE["main"] = _build_main()
    return _CACHE["qkv"], _CACHE["main"]


def _swizzle(wT, nb):
    """[K, N] (contraction-major) -> SBUF image [128, (N/nb) * (K/128) * nb]:
    out[p, b, a, s] = wT[a*128 + p, b*nb + s], flattened over (b, a, s)."""
    K, N = wT.shape
    kt, npb = K // P, N // nb
    return np.ascontiguousarray(
        wT.reshape(kt, P, npb, nb).transpose(1, 2, 0, 3).reshape(P, npb * kt * nb)
    )


def _prep(x, fc1_w, fc1_b, fc2_w, fc2_b, q_w, q_b, k_w, k_b, v_w, v_b, o_w, o_b):
    f32 = np.float32
    xT_bf = np.ascontiguousarray(np.asarray(x, f32).T).astype(BF_NP)
    wq_t = _swizzle(np.asarray(q_w, f32).T.astype(BF_NP), 512)
    wk_t = _swizzle(np.asarray(k_w, f32).T.astype(BF_NP), 512)
    wv_t = _swizzle(np.asarray(v_w, f32).T.astype(BF_NP), 512)
    wo_t = _swizzle(np.asarray(o_w, f32).T.astype(BF_NP), 512)
    w1_t = _swizzle(np.asarray(fc1_w, f32).T.astype(BF_NP), 512)
    w2T = np.asarray(fc2_w, f32).T.astype(BF_NP)                   # [EXP, HID]
    # fc2 stream layout [p, n(4), kq(16), kk(4), s(512)]:
    # element = w2T[(kq*4+kk)*128 + p, n*512 + s]
    w2_t = np.ascontiguousarray(
        w2T.reshape(16, 4, P, 4, 512).transpose(2, 3, 0, 1, 4).reshape(P, -1)
    )
    qb2 = np.ascontiguousarray(np.asarray(q_b, f32).reshape(HT, P).T)
    kb2 = np.ascontiguousarray(np.asarray(k_b, f32).reshape(HT, P).T)
    b12 = np.ascontiguousarray(np.asarray(fc1_b, f32).reshape(ET, P).T)

    xsh_imgs = []
    for c in range(NCORES):
        xc = np.ascontiguousarray(xT_bf[:, c * SSH:(c + 1) * SSH])
        xsh_imgs.append(np.ascontiguousarray(
            xc.reshape(HT, P, SSH).transpose(1, 0, 2).reshape(P, -1)
        ))
    host_add = (
        np.asarray(fc2_b, f32)
        + np.asarray(o_b, f32)
        + np.asarray(o_w, f32) @ np.asarray(v_b, f32)
    )
    return {
        "xsh": xsh_imgs, "wq": wq_t, "wk": wk_t, "wv": wv_t, "wo": wo_t,
        "w1": w1_t, "w2": w2_t, "qb2": qb2, "kb2": kb2, "b12": b12,
        "host_add": host_add,
    }


def run(trace=False, tmpdir=None, **inputs):
    nc1, nc2 = _get_ncs()
    pp = _prep(**inputs)
    if tmpdir:
        os.makedirs(tmpdir + "/l1", exist_ok=True)
        os.makedirs(tmpdir + "/l2", exist_ok=True)
    in1 = [{
        "xsh": pp["xsh"][c], "wq": pp["wq"], "wk": pp["wk"], "wv": pp["wv"],
        "qb2": pp["qb2"], "kb2": pp["kb2"],
    } for c in range(NCORES)]
    res1 = run_bass_kernel_spmd(
        nc1, in1, core_ids=list(range(NCORES)), trace=trace,
        tmpdir=(tmpdir + "/l1") if tmpdir else None,
    )
    kT_all = np.concatenate([res1.results[c]["kT_o"] for c in range(NCORES)], axis=0)
    v_all = np.concatenate([res1.results[c]["v_o"] for c in range(NCORES)], axis=0)

    in2 = [{
        "xsh": pp["xsh"][c], "qTi": res1.results[c]["qT_o"],
        "kT_all": kT_all, "v_all": v_all,
        "wo": pp["wo"], "w1": pp["w1"], "w2": pp["w2"], "b12": pp["b12"],
    } for c in range(NCORES)]
    res2 = run_bass_kernel_spmd(
        nc2, in2, core_ids=list(range(NCORES)), trace=trace,
        tmpdir=(tmpdir + "/l2") if tmpdir else None,
    )
    outp = np.concatenate(
        [res2.results[c]["out"] for c in range(NCORES)], axis=0
    ) + pp["host_add"][None, :]
    return outp.astype(np.float32), (res1, res2)


def kernel(**inputs):
    outp, _ = run(trace=False, **inputs)
    return outp


# revision 32
# speedup vs baseline: 1.0205x; 1.0171x over previous
"""Trainium2 Bass kernel for nn_DistributedExpert (dense transformer expert).

Computes, for x [4096, 2048]:
    h   = gelu(x @ fc1_w.T + fc1_b) @ fc2_w.T + fc2_b          (MLP branch)
    q/k/v = x @ {q,k,v}_w.T + b
    attn  = softmax(q @ k.T / sqrt(2048))
    out = (attn @ v) @ o_w.T + o_b + h

Distribution over 8 NeuronCores — everything is sequence-sharded (each core
owns 512 rows of x and of the output). Two collective-free launches:

  Launch 1: each core computes q/k/v (transposed layouts) for its rows.
  Host:     gathers the k/v shards (this replaces an on-device AllGather —
            measured: having ANY collective in the NEFF slows every matmul
            by ~21%, 216 -> 263 ns, so the gather is done on the host).
  Launch 2: MLP (full weights streamed, no expert sharding) + attention +
            output projection + combine.

  - Softmax uses the no-max-subtraction form (scores are O(1)); row sums are
    built with ones-matmuls on the transposed attention layout and the
    normalization is applied after the output projection.
  - Biases with an easy per-partition broadcast (q_b, k_b, fc1_b) are added
    on-device; v_b/o_b/fc2_b contributions are mathematically exact constant
    row-vectors, added on the host.

Matmuls run in bf16 (fp32 PSUM accumulation). All streamed weights are
pre-swizzled on the host into their exact SBUF image ([128 partitions x
contiguous free bytes]) so every weight DMA is a full-bandwidth linear copy.
"""

import os
import sys

sys.path.insert(0, "/opt/trn_rl_repo")

import numpy as np
import ml_dtypes

import concourse.bass as bass
import concourse.mybir as mybir
import concourse.tile as tile
from concourse import bacc
from concourse.bass_utils import run_bass_kernel_spmd

SEQ = 4096
HID = 2048
EXP = 8192
NCORES = 8
SSH = SEQ // NCORES   # 512 sequence rows per core
P = 128

HT = HID // P        # 16 hidden tiles
ET = EXP // P        # 64 expert tiles
ST = SSH // P        # 4 local-seq tiles
GT = SEQ // P        # 32 global-seq tiles

F32 = mybir.dt.float32
BF16 = mybir.dt.bfloat16
AF = mybir.ActivationFunctionType
BF_NP = ml_dtypes.bfloat16

_CACHE = {}


def _wslice(w_ap, blk, ntiles):
    # [128, ntiles, 512] SBUF-image slice for output-block `blk`
    sz = ntiles * 512
    return w_ap[:, blk * sz:(blk + 1) * sz].rearrange("p (a s) -> p a s", a=ntiles)


def _build_qkv():
    nc = bacc.Bacc("TRN2", target_bir_lowering=False, debug=False, num_devices=NCORES)
    xsh = nc.dram_tensor("xsh", [P, HT * SSH], BF16, kind="ExternalInput").ap()
    wq = nc.dram_tensor("wq", [P, 4 * HT * 512], BF16, kind="ExternalInput").ap()
    wk = nc.dram_tensor("wk", [P, 4 * HT * 512], BF16, kind="ExternalInput").ap()
    wv = nc.dram_tensor("wv", [P, 4 * HT * 512], BF16, kind="ExternalInput").ap()
    qb2 = nc.dram_tensor("qb2", [P, HT], F32, kind="ExternalInput").ap()
    kb2 = nc.dram_tensor("kb2", [P, HT], F32, kind="ExternalInput").ap()
    qT_o = nc.dram_tensor("qT_o", [P, HT * SSH], BF16, kind="ExternalOutput").ap()
    kT_o = nc.dram_tensor("kT_o", [P, HT * SSH], BF16, kind="ExternalOutput").ap()
    v_o = nc.dram_tensor("v_o", [P, ST * HID], BF16, kind="ExternalOutput").ap()

    with tile.TileContext(nc) as tc:
        with tc.tile_pool(name="const", bufs=1) as constp, \
             tc.tile_pool(name="st", bufs=1) as st, \
             tc.tile_pool(name="ws", bufs=4) as ws, \
             tc.tile_pool(name="ps", bufs=4, space="PSUM") as ps:
            qb_s = constp.tile([P, HT], F32)
            kb_s = constp.tile([P, HT], F32)
            nc.sync.dma_start(qb_s[:], qb2[:])
            nc.sync.dma_start(kb_s[:], kb2[:])
            xs = st.tile([P, HT, SSH], BF16)
            xsr = xsh.rearrange("p (a s) -> p a s", a=HT)
            # interleave x chunks with the first weight group's chunks so the
            # first matmul chain unblocks after ~1MB instead of ~4.2MB
            wt0 = ws.tile([P, HT, 512], BF16, name="wtile")
            w0sl = _wslice(wq, 0, HT)
            for cch in range(4):
                nc.sync.dma_start(
                    xs[:, cch * 4:(cch + 1) * 4, :], xsr[:, cch * 4:(cch + 1) * 4, :]
                )
                nc.sync.dma_start(
                    wt0[:, cch * 4:(cch + 1) * 4, :], w0sl[:, cch * 4:(cch + 1) * 4, :]
                )
            qT = st.tile([P, HT, SSH], BF16)
            kT = st.tile([P, HT, SSH], BF16)
            v_s = st.tile([P, ST, HID], BF16)

            for dst, dst_o, w_ap, bias in (
                (qT, qT_o, wq, qb_s), (kT, kT_o, wk, kb_s)
            ):
                for g in range(4):
                    if w_ap is wq and g == 0:
                        wt = wt0
                    else:
                        wt = ws.tile([P, HT, 512], BF16, name="wtile")
                        nc.sync.dma_start(wt[:], _wslice(w_ap, g, HT))
                    for m in range(4):
                        pt = ps.tile([P, SSH], F32, name="pA")
                        for k in range(HT):
                            nc.tensor.matmul(
                                pt[:], wt[:, k, m * P:(m + 1) * P], xs[:, k, :],
                                start=(k == 0), stop=(k == HT - 1),
                            )
                        nc.scalar.activation(
                            dst[:, g * 4 + m, :], pt[:], AF.Identity,
                            bias=bias[:, g * 4 + m:g * 4 + m + 1],
                        )
                    # stream this group's 4 tiles out while the next computes
                    nc.sync.dma_start(
                        _wslice(dst_o, g, 4), dst[:, g * 4:(g + 1) * 4, :]
                    )
            # v in natural layout [s_local, hid]  (v_b folded on host)
            for n in range(4):
                wt = ws.tile([P, HT, 512], BF16, name="wtile")
                nc.sync.dma_start(wt[:], _wslice(wv, n, HT))
                for m in range(ST):
                    pt = ps.tile([P, SSH], F32, name="pA")
                    for k in range(HT):
                        nc.tensor.matmul(
                            pt[:], xs[:, k, m * P:(m + 1) * P], wt[:, k, :],
                            start=(k == 0), stop=(k == HT - 1),
                        )
                    nc.vector.tensor_copy(v_s[:, m, n * 512:(n + 1) * 512], pt[:])
                nc.sync.dma_start(
                    v_o.rearrange("p (a s) -> p a s", a=ST)[:, :, n * 512:(n + 1) * 512],
                    v_s[:, :, n * 512:(n + 1) * 512],
                )
    nc.compile()
    return nc


def _build_main():
    nc = bacc.Bacc("TRN2", target_bir_lowering=False, debug=False, num_devices=NCORES)
    EXPSCALE = 1.0 / float(np.sqrt(np.float32(HID)))

    xsh = nc.dram_tensor("xsh", [P, HT * SSH], BF16, kind="ExternalInput").ap()
    qTi = nc.dram_tensor("qTi", [P, HT * SSH], BF16, kind="ExternalInput").ap()
    kT_all = nc.dram_tensor("kT_all", [NCORES * P, HT * SSH], BF16,
                            kind="ExternalInput").ap()
    v_all = nc.dram_tensor("v_all", [NCORES * P, ST * HID], BF16,
                           kind="ExternalInput").ap()
    wo = nc.dram_tensor("wo", [P, 4 * HT * 512], BF16, kind="ExternalInput").ap()
    w1 = nc.dram_tensor("w1", [P, 16 * HT * 512], BF16, kind="ExternalInput").ap()
    w2 = nc.dram_tensor("w2", [P, 4 * ET * 512], BF16, kind="ExternalInput").ap()
    b12 = nc.dram_tensor("b12", [P, ET], F32, kind="ExternalInput").ap()
    out = nc.dram_tensor("out", [SSH, HID], F32, kind="ExternalOutput").ap()

    with tile.TileContext(nc) as tc:
        with tc.tile_pool(name="const", bufs=1) as constp, \
             tc.tile_pool(name="persist", bufs=1) as persist:
            ones = constp.tile([P, 1], BF16)
            nc.vector.memset(ones[:], 1.0)
            b1_s = constp.tile([P, ET], F32)
            nc.sync.dma_start(b1_s[:], b12[:])

            xs = persist.tile([P, HT, SSH], BF16)
            xsr = xsh.rearrange("p (a s) -> p a s", a=HT)
            qT = persist.tile([P, HT, SSH], BF16)
            kb0 = persist.tile([P, HT, SSH], BF16)

            # ======== MLP, sequence-sharded, full weights ========
            # Two expert-halves of 4096 so gelu(fc1) [e, s] needs only a
            # half-size buffer; the second fc2 pass accumulates via DVE add.
            scope_mlp = nc.named_scope("mlp"); scope_mlp.__enter__()
            h_sb = persist.tile([P, ST, HID], F32)  # local MLP output (f32)
            EHALF = ET // 2
            with tc.tile_pool(name="w1B", bufs=2) as w1B, \
                 tc.tile_pool(name="w2B", bufs=2) as w2B, \
                 tc.tile_pool(name="gB", bufs=1) as gB, \
                 tc.tile_pool(name="psB", bufs=4, space="PSUM") as psB, \
                 tc.tile_pool(name="psB2", bufs=1, space="PSUM") as psB2:
                for half in range(2):
                    g1 = gB.tile([P, EHALF, SSH], BF16, name="g1")
                    for eg in range(8):
                        ego = half * 8 + eg
                        w1g = w1B.tile([P, HT, 512], BF16, name="w1g")
                        w1sl = _wslice(w1, ego, HT)
                        if half == 0 and eg == 0:
                            # critical path: interleave x chunks with the first
                            # fc1 weight group so matmuls start after ~1MB
                            for cch in range(4):
                                nc.sync.dma_start(
                                    xs[:, cch * 4:(cch + 1) * 4, :],
                                    xsr[:, cch * 4:(cch + 1) * 4, :],
                                )
                                nc.sync.dma_start(
                                    w1g[:, cch * 4:(cch + 1) * 4, :],
                                    w1sl[:, cch * 4:(cch + 1) * 4, :],
                                )
                        else:
                            nc.sync.dma_start(w1g[:], w1sl)
                        if half == 0 and eg == 1:
                            # attention inputs, needed much later — loaded
                            # behind the first weight groups
                            nc.sync.dma_start(
                                qT[:], qTi.rearrange("p (a s) -> p a s", a=HT)
                            )
                            nc.sync.dma_start(
                                kb0[:],
                                kT_all[0:P, :].rearrange("p (a s) -> p a s", a=HT),
                            )
                        for m in range(4):
                            pt = psB.tile([P, SSH], F32, name="pB1")
                            for k in range(HT):
                                nc.tensor.matmul(
                                    pt[:], w1g[:, k, m * P:(m + 1) * P], xs[:, k, :],
                                    start=(k == 0), stop=(k == HT - 1),
                                )
                            nc.scalar.activation(
                                g1[:, eg * 4 + m, :], pt[:], AF.Gelu,
                                bias=b1_s[:, ego * 4 + m:ego * 4 + m + 1],
                            )
                    for n in range(4):
                        pts = [psB2.tile([P, 512], F32, name=f"pB2{m}")
                               for m in range(ST)]
                        for qtr in range(2):
                            base = (n * 16 + half * 8 + qtr * 4) * 2048
                            w2g = w2B.tile([P, 16, 512], BF16, name="w2g")
                            nc.sync.dma_start(
                                w2g[:],
                                w2[:, base:base + 8192].rearrange(
                                    "p (a s) -> p a s", a=16
                                ),
                            )
                            for m in range(ST):
                                for kk in range(16):
                                    k = qtr * 16 + kk
                                    nc.tensor.matmul(
                                        pts[m][:], g1[:, k, m * P:(m + 1) * P],
                                        w2g[:, kk, :],
                                        start=(k == 0), stop=(k == EHALF - 1),
                                    )
                        for m in range(ST):
                            if half == 0:
                                nc.vector.tensor_copy(
                                    h_sb[:, m, n * 512:(n + 1) * 512], pts[m][:]
                                )
                            else:
                                nc.vector.tensor_add(
                                    h_sb[:, m, n * 512:(n + 1) * 512],
                                    h_sb[:, m, n * 512:(n + 1) * 512], pts[m][:],
                                )
            scope_mlp.__exit__(None, None, None)

            # ================= attention =================
            scope_att = nc.named_scope("attn"); scope_att.__enter__()
            attT = persist.tile([P, HT, SSH], BF16)     # (E @ v).T
            recip = persist.tile([P, ST], F32)

            kC_cm = tc.tile_pool(name="kC", bufs=2)
            kC = kC_cm.__enter__()
            ots = {}
            with tc.tile_pool(name="aC", bufs=1) as aC, \
                 tc.tile_pool(name="vC", bufs=2) as vC, \
                 tc.tile_pool(name="psC", bufs=3, space="PSUM") as psC, \
                 tc.tile_pool(name="psR", bufs=1, space="PSUM") as psR, \
                 tc.tile_pool(name="psV", bufs=1, space="PSUM") as psV:
                attnT = aC.tile([P, GT, SSH], BF16)   # exp(scores).T (unnormalized)
                for mb in range(NCORES):
                    if mb == 0:
                        kb = kb0
                    else:
                        kb = kC.tile([P, HT, SSH], BF16, name="kb")
                        nc.sync.dma_start(
                            kb[:],
                            kT_all[mb * P:(mb + 1) * P, :].rearrange(
                                "p (a s) -> p a s", a=HT
                            ),
                        )
                    for mm in range(4):
                        pt = psC.tile([P, SSH], F32, name="pC")
                        for k in range(HT):
                            nc.tensor.matmul(
                                pt[:], kb[:, k, mm * P:(mm + 1) * P], qT[:, k, :],
                                start=(k == 0), stop=(k == HT - 1),
                            )
                        nc.scalar.activation(
                            attnT[:, mb * 4 + mm, :], pt[:], AF.Exp, scale=EXPSCALE
                        )

                # prefetch the first o-projection weight block through the
                # same pool slots the kb tiles used
                ot0 = kC.tile([P, HT, 512], BF16, name="kb")
                nc.sync.dma_start(ot0[:], _wslice(wo, 0, HT))
                ots[0] = ot0

                # E @ v, transposed: attT[h, s_local]
                for g in range(4):
                    pts = [psV.tile([P, SSH], F32, name=f"pV{m}") for m in range(4)]
                    for q4 in range(2):
                        vt = vC.tile([P, 16, 512], BF16, name="vt")
                        for j in range(4):
                            rb = q4 * 4 + j
                            nc.sync.dma_start(
                                vt[:, j * 4:(j + 1) * 4, :],
                                v_all[rb * P:(rb + 1) * P, :].rearrange(
                                    "p (a s) -> p a s", a=ST
                                )[:, :, g * 512:(g + 1) * 512],
                            )
                        for m in range(4):
                            for kk in range(16):
                                k = q4 * 16 + kk
                                nc.tensor.matmul(
                                    pts[m][:], vt[:, kk, m * P:(m + 1) * P],
                                    attnT[:, k, :],
                                    start=(k == 0), stop=(k == GT - 1),
                                )
                    for m in range(4):
                        nc.vector.tensor_copy(attT[:, g * 4 + m, :], pts[m][:])

                # softmax row sums via ones-matmuls
                prs = psR.tile([P, ST], F32)
                for k in range(GT):
                    for m2 in range(ST):
                        nc.tensor.matmul(
                            prs[:, m2:m2 + 1], attnT[:, k, m2 * P:(m2 + 1) * P],
                            ones[:], start=(k == 0), stop=(k == GT - 1),
                        )
                nc.vector.reciprocal(recip[:], prs[:])
            scope_att.__exit__(None, None, None)

            # ============ output projection + combine ============
            scope_o = nc.named_scope("oproj"); scope_o.__enter__()
            with tc.tile_pool(name="evD", bufs=4) as evD, \
                 tc.tile_pool(name="psD", bufs=4, space="PSUM") as psD:
                for n in range(4):
                    if n in ots:
                        ot = ots[n]
                    else:
                        ot = kC.tile([P, HT, 512], BF16, name="kb")
                        nc.sync.dma_start(ot[:], _wslice(wo, n, HT))
                    for m in range(ST):
                        pt = psD.tile([P, 512], F32, name="pD")
                        for k in range(HT):
                            nc.tensor.matmul(
                                pt[:], attT[:, k, m * P:(m + 1) * P], ot[:, k, :],
                                start=(k == 0), stop=(k == HT - 1),
                            )
                        ev = evD.tile([P, 512], F32, name="evD")
                        nc.vector.tensor_scalar_mul(ev[:], pt[:], recip[:, m:m + 1])
                        nc.vector.tensor_add(
                            ev[:], ev[:], h_sb[:, m, n * 512:(n + 1) * 512]
                        )
                        nc.sync.dma_start(
                            out[m * P:(m + 1) * P, n * 512:(n + 1) * 512], ev[:]
                        )
            scope_o.__exit__(None, None, None)
            kC_cm.__exit__(None, None, None)

    nc.compile()
    return nc


def _get_ncs():
    if "qkv" not in _CACHE:
        _CACHE["qkv"] = _build_qkv()
        _CACHE["main"] = _build_main()
    return _CACHE["qkv"], _CACHE["main"]


def _swizzle(wT, nb):
    """[K, N] (contraction-major) -> SBUF image [128, (N/nb) * (K/128) * nb]:
    out[p, b, a, s] = wT[a*128 + p, b*nb + s], flattened over (b, a, s)."""
    K, N = wT.shape
    kt, npb = K // P, N // nb
    return np.ascontiguousarray(
        wT.reshape(kt, P, npb, nb).transpose(1, 2, 0, 3).reshape(P, npb * kt * nb)
    )


def _prep(x, fc1_w, fc1_b, fc2_w, fc2_b, q_w, q_b, k_w, k_b, v_w, v_b, o_w, o_b):
    f32 = np.float32
    xT_bf = np.ascontiguousarray(np.asarray(x, f32).T).astype(BF_NP)
    wq_t = _swizzle(np.asarray(q_w, f32).T.astype(BF_NP), 512)
    wk_t = _swizzle(np.asarray(k_w, f32).T.astype(BF_NP), 512)
    wv_t = _swizzle(np.asarray(v_w, f32).T.astype(BF_NP), 512)
    wo_t = _swizzle(np.asarray(o_w, f32).T.astype(BF_NP), 512)
    w1_t = _swizzle(np.asarray(fc1_w, f32).T.astype(BF_NP), 512)
    w2T = np.asarray(fc2_w, f32).T.astype(BF_NP)                   # [EXP, HID]
    # fc2 stream layout [p, n(4), kq(16), kk(4), s(512)]:
    # element = w2T[(kq*4+kk)*128 + p, n*512 + s]
    w2_t = np.ascontiguousarray(
        w2T.reshape(16, 4, P, 4, 512).transpose(2, 3, 0, 1, 4).reshape(P, -1)
    )
    qb2 = np.ascontiguousarray(np.asarray(q_b, f32).reshape(HT, P).T)
    kb2 = np.ascontiguousarray(np.asarray(k_b, f32).reshape(HT, P).T)
    b12 = np.ascontiguousarray(np.asarray(fc1_b, f32).reshape(ET, P).T)

    xsh_imgs = []
    for c in range(NCORES):
        xc = np.ascontiguousarray(xT_bf[:, c * SSH:(c + 1) * SSH])
        xsh_imgs.append(np.ascontiguousarray(
            xc.reshape(HT, P, SSH).transpose(1, 0, 2).reshape(P, -1)
        ))
    host_add = (
        np.asarray(fc2_b, f32)
        + np.asarray(o_b, f32)
        + np.asarray(o_w, f32) @ np.asarray(v_b, f32)
    )
    return {
        "xsh": xsh_imgs, "wq": wq_t, "wk": wk_t, "wv": wv_t, "wo": wo_t,
        "w1": w1_t, "w2": w2_t, "qb2": qb2, "kb2": kb2, "b12": b12,
        "host_add": host_add,
    }


def run(trace=False, tmpdir=None, **inputs):
    nc1, nc2 = _get_ncs()
    pp = _prep(**inputs)
    if tmpdir:
        os.makedirs(tmpdir + "/l1", exist_ok=True)
        os.makedirs(tmpdir + "/l2", exist_ok=True)
    in1 = [{
        "xsh": pp["xsh"][c], "wq": pp["wq"], "wk": pp["wk"], "wv": pp["wv"],
        "qb2": pp["qb2"], "kb2": pp["kb2"],
    } for c in range(NCORES)]
    res1 = run_bass_kernel_spmd(
        nc1, in1, core_ids=list(range(NCORES)), trace=trace,
        tmpdir=(tmpdir + "/l1") if tmpdir else None,
    )
    kT_all = np.concatenate([res1.results[c]["kT_o"] for c in range(NCORES)], axis=0)
    v_all = np.concatenate([res1.results[c]["v_o"] for c in range(NCORES)], axis=0)

    in2 = [{
        "xsh": pp["xsh"][c], "qTi": res1.results[c]["qT_o"],
        "kT_all": kT_all, "v_all": v_all,
        "wo": pp["wo"], "w1": pp["w1"], "w2": pp["w2"], "b12": pp["b12"],
    } for c in range(NCORES)]
    res2 = run_bass_kernel_spmd(
        nc2, in2, core_ids=list(range(NCORES)), trace=trace,
        tmpdir=(tmpdir + "/l2") if tmpdir else None,
    )
    outp = np.concatenate(
        [res2.results[c]["out"] for c in range(NCORES)], axis=0
    ) + pp["host_add"][None, :]
    return outp.astype(np.float32), (res1, res2)


def kernel(**inputs):
    outp, _ = run(trace=False, **inputs)
    return outp
